# Initial kernel scaffold
#
"""Trainium2 Bass kernel for nn_Position_AM_Module_3D.

Reference computation (B=2, C=256, INTER=64, N=T*H*W=8192):
    q = Wq @ xf + bq          [B, N, 64]   (as Q [64, N] per batch)
    k = Wk @ xf + bk          [B, 64, N]
    v = Wv @ xf + bv          [B, 256, N]
    att = softmax(q @ k, -1)  [B, N, N]    (never materialized)
    out = v @ att^T + x       [B, 256, N]

Sharding: 8 cores = (batch b in 0..1) x (query quarter qc in 0..3).
Each core gets the full xf for its batch plus its 2048-query window, and
computes out[:, window] with a flash-attention-style k-loop:
  S^T tile [128k, 512q] = K_slice^T @ Q_chunk  (PE, f32r)
  e = exp(S^T)                                  (ACT, no max subtraction --
                                                 |energy| < ~60 << 88, safe in fp32)
  av[c,q]  += V^T_slice^T @ e                   (PE accumulate in PSUM)
  den[1,q] += ones^T @ e                        (PE accumulate in PSUM)
then out = av * (1/den) + x_window.
"""

import numpy as np

B, C, INTER = 2, 256, 64
T3, H3, W3 = 8, 32, 32
N = T3 * H3 * W3          # 8192
NCORES = 8
QCH = N // 4              # 2048 queries per core
QT = 512                  # q-chunk (one PSUM bank of fp32)
KT = 128                  # k-tile (PE partition dim)
NKT = N // KT             # 64 k-tiles
GK = 2                    # k-tiles per exp group ([128, 1024] ACTIVATE)
NG = NKT // GK            # 32 groups

_prog_cache = {}


def _build_program(mm_dtype_name="float32r"):
    from contextlib import ExitStack

    import concourse.bass as bass
    import concourse.mybir as mybir
    import concourse.tile as tile

    f32 = mybir.dt.float32
    mmdt = getattr(mybir.dt, mm_dtype_name)

    def mm(ap):
        return ap.bitcast(mmdt)

    nc = bass.Bass()

    xf = nc.declare_dram_parameter("xf", [C, N], f32, isOutput=False)
    xq = nc.declare_dram_parameter("xq", [C, QCH], f32, isOutput=False)
    wkT = nc.declare_dram_parameter("wkT", [C, INTER], f32, isOutput=False)
    wqT = nc.declare_dram_parameter("wqT", [C, INTER], f32, isOutput=False)
    wvT = nc.declare_dram_parameter("wvT", [C, C], f32, isOutput=False)
    bk = nc.declare_dram_parameter("bk", [INTER, 1], f32, isOutput=False)
    bq = nc.declare_dram_parameter("bq", [INTER, 1], f32, isOutput=False)
    bv = nc.declare_dram_parameter("bv", [1, C], f32, isOutput=False)
    out = nc.declare_dram_parameter("out", [C, QCH], f32, isOutput=True)

    Exp = mybir.ActivationFunctionType.Exp
    add_op = mybir.AluOpType.add

    with ExitStack() as ctx, tile.TileContext(nc) as tc:
        singles = ctx.enter_context(tc.tile_pool(name="singles", bufs=1))

        # persistent SBUF tensors
        wkT_sb = singles.tile([128, 2, INTER], f32)
        wqT_sb = singles.tile([128, 2, INTER], f32)
        wvT_sb = singles.tile([128, 2, C], f32)
        bk_sb = singles.tile([INTER, 1], f32)
        bq_sb = singles.tile([INTER, 1], f32)
        bv_row = singles.tile([1, C], f32)
        bv_bc = singles.tile([128, C], f32)
        ones_col = singles.tile([128, 1], f32)
        ones_row = singles.tile([1, 128], f32)
        K_sb = singles.tile([INTER, N], f32)
        Q_sb = singles.tile([INTER, QCH], f32)
        VT_sb = singles.tile([128, NKT, C], f32)
        xq_sb = singles.tile([128, 2, QCH], f32)

        nc.vector.memset(ones_col, 1.0)
        nc.vector.memset(ones_row, 1.0)

        for ci in range(2):
            nc.sync.dma_start(out=wkT_sb[:, ci, :], in_=wkT[ci * 128:(ci + 1) * 128, :])
            nc.sync.dma_start(out=wqT_sb[:, ci, :], in_=wqT[ci * 128:(ci + 1) * 128, :])
            nc.sync.dma_start(out=wvT_sb[:, ci, :], in_=wvT[ci * 128:(ci + 1) * 128, :])
            nc.sync.dma_start(out=xq_sb[:, ci, :], in_=xq[ci * 128:(ci + 1) * 128, :])
        nc.sync.dma_start(out=bk_sb, in_=bk)
        nc.sync.dma_start(out=bq_sb, in_=bq)
        nc.sync.dma_start(out=bv_row, in_=bv)
        # broadcast bv across partitions (SBUF->SBUF DMA, src partition step 0)
        bv_src = bass.AP(
            tensor=bv_row.tensor,
            offset=bv_row.offset,
            ap=[[0, 128]] + list(bv_row.ap[1:]),
        )
        nc.sync.dma_start(out=bv_bc, in_=bv_src)

        # ---------------- Phase A: projections ----------------
        with tc.tile_pool(name="xfp", bufs=1) as xfp, \
             tc.tile_pool(name="ppsum", bufs=4, space="PSUM") as ppsum:
            xf_sb = xfp.tile([128, 2, N], f32)
            for ci in range(2):
                for s in range(8):
                    nc.sync.dma_start(
                        out=xf_sb[:, ci, s * 1024:(s + 1) * 1024],
                        in_=xf[ci * 128:(ci + 1) * 128, s * 1024:(s + 1) * 1024],
                    )

            # K = Wk @ xf + bk   -> K_sb [64, N]
            for nt in range(N // 512):
                pk = ppsum.tile([INTER, 512], f32, tag="pk")
                for ci in range(2):
                    nc.tensor.matmul(
                        pk,
                        mm(wkT_sb[:, ci, :]),
                        mm(xf_sb[:, ci, nt * 512:(nt + 1) * 512]),
                        start=(ci == 0),
                        stop=(ci == 1),
                    )
                nc.vector.tensor_scalar(
                    K_sb[:, nt * 512:(nt + 1) * 512], pk, bk_sb, None, add_op
                )

            # V^T = xf^T @ Wv^T + bv -> VT_sb [128, jt, C]
            for jt in range(NKT):
                pv = ppsum.tile([128, C], f32, tag="pv")
                for ci in range(2):
                    nc.tensor.matmul(
                        pv,
                        mm(xf_sb[:, ci, jt * 128:(jt + 1) * 128]),
                        mm(wvT_sb[:, ci, :]),
                        start=(ci == 0),
                        stop=(ci == 1),
                    )
                nc.vector.tensor_tensor(VT_sb[:, jt, :], pv, bv_bc, add_op)

            # Q = Wq @ xq + bq -> Q_sb [64, QCH]
            for qt in range(QCH // 512):
                pq = ppsum.tile([INTER, 512], f32, tag="pk")
                for ci in range(2):
                    nc.tensor.matmul(
                        pq,
                        mm(wqT_sb[:, ci, :]),
                        mm(xq_sb[:, ci, qt * 512:(qt + 1) * 512]),
                        start=(ci == 0),
                        stop=(ci == 1),
                    )
                nc.vector.tensor_scalar(
                    Q_sb[:, qt * 512:(qt + 1) * 512], pq, bq_sb, None, add_op
                )

        # ---------------- Phase B: attention ----------------
        with tc.tile_pool(name="epool", bufs=2) as epool, \
             tc.tile_pool(name="spsum", bufs=2, space="PSUM") as spsum, \
             tc.tile_pool(name="avpsum", bufs=1, space="PSUM") as avpsum, \
             tc.tile_pool(name="dpsum", bufs=1, space="PSUM") as dpsum, \
             tc.tile_pool(name="misc", bufs=2) as misc, \
             tc.tile_pool(name="outp", bufs=4) as outp:

            for qc in range(QCH // QT):
                q_rhs = mm(Q_sb[:, qc * QT:(qc + 1) * QT])
                av = avpsum.tile([128, 2 * QT], f32, tag="av")
                dps = dpsum.tile([1, QT], f32, tag="d")

                # software-pipelined: S(g+1)/exp(g+1) issued before AV(g)
                def s_group(g):
                    sps = spsum.tile([128, GK * QT], f32, tag="s")
                    for t in range(GK):
                        kt = g * GK + t
                        nc.tensor.matmul(
                            sps[:, t * QT:(t + 1) * QT],
                            mm(K_sb[:, kt * KT:(kt + 1) * KT]),
                            q_rhs,
                            start=True,
                            stop=True,
                        )
                    e_t = epool.tile([128, GK * QT], f32, tag="e")
                    nc.scalar.activation(e_t, sps, Exp)
                    return e_t

                def av_group(g, e_t):
                    for t in range(GK):
                        kt = g * GK + t
                        er = mm(e_t[:, t * QT:(t + 1) * QT])
                        first = kt == 0
                        last = kt == NKT - 1
                        nc.tensor.matmul(
                            av[:, 0:QT], mm(VT_sb[:, kt, 0:128]), er,
                            start=first, stop=last,
                        )
                        nc.tensor.matmul(
                            av[:, QT:2 * QT], mm(VT_sb[:, kt, 128:256]), er,
                            start=first, stop=last,
                        )
                        nc.tensor.matmul(
                            dps, mm(ones_col), er,
                            start=first, stop=last,
                        )

                e_prev = s_group(0)
                for g in range(NG):
                    e_next = s_group(g + 1) if g + 1 < NG else None
                    av_group(g, e_prev)
                    e_prev = e_next

                # epilogue: out = av / den + xq
                recip = misc.tile([1, QT], f32, tag="recip")
                nc.vector.reciprocal(recip, dps)
                bsb = misc.tile([128, QT], f32, tag="bsb")
                r_src = bass.AP(
                    tensor=recip.tensor,
                    offset=recip.offset,
                    ap=[[0, 128]] + list(recip.ap[1:]),
                )
                nc.sync.dma_start(out=bsb, in_=r_src)
                for ch in range(2):
                    o_t = outp.tile([128, QT], f32, tag="o")
                    nc.vector.tensor_mul(o_t, av[:, ch * QT:(ch + 1) * QT], bsb)
                    nc.vector.tensor_add(
                        o_t, o_t, xq_sb[:, ch, qc * QT:(qc + 1) * QT]
                    )
                    nc.sync.dma_start(
                        out=out[ch * 128:(ch + 1) * 128, qc * QT:(qc + 1) * QT],
                        in_=o_t,
                    )

    return nc


def _get_program(mm_dtype_name="float32r"):
    if mm_dtype_name not in _prog_cache:
        _prog_cache[mm_dtype_name] = _build_program(mm_dtype_name)
    return _prog_cache[mm_dtype_name]


def _make_in_maps(x, Wq, bq, Wk, bk, Wv, bv):
    x = np.ascontiguousarray(np.asarray(x, np.float32))
    xf = x.reshape(B, C, N)
    wkT = np.ascontiguousarray(np.asarray(Wk, np.float32).T)
    wqT = np.ascontiguousarray(np.asarray(Wq, np.float32).T)
    wvT = np.ascontiguousarray(np.asarray(Wv, np.float32).T)
    bkc = np.ascontiguousarray(np.asarray(bk, np.float32).reshape(INTER, 1))
    bqc = np.ascontiguousarray(np.asarray(bq, np.float32).reshape(INTER, 1))
    bvc = np.ascontiguousarray(np.asarray(bv, np.float32).reshape(1, C))
    in_maps = []
    for core in range(NCORES):
        b, qc = divmod(core, NCORES // B)
        sl = slice(qc * QCH, (qc + 1) * QCH)
        in_maps.append({
            "xf": np.ascontiguousarray(xf[b]),
            "xq": np.ascontiguousarray(xf[b][:, sl]),
            "wkT": wkT, "wqT": wqT, "wvT": wvT,
            "bk": bkc, "bq": bqc, "bv": bvc,
        })
    return in_maps


def _gather(results):
    full = np.empty((B, C, N), np.float32)
    for core in range(NCORES):
        b, qc = divmod(core, NCORES // B)
        full[b][:, qc * QCH:(qc + 1) * QCH] = results[core]["out"]
    return full.reshape(B, C, T3, H3, W3)


def run(x, Wq, bq, Wk, bk, Wv, bv, trace=False, **spmd_kwargs):
    """Run the kernel; returns (output, BassKernelResults)."""
    from concourse.bass_utils import run_bass_kernel_spmd

    nc = _get_program()
    in_maps = _make_in_maps(x, Wq, bq, Wk, bk, Wv, bv)
    res = run_bass_kernel_spmd(
        nc, in_maps, list(range(NCORES)), trace=trace, **spmd_kwargs
    )
    return _gather(res.results), res


def kernel(x, Wq, bq, Wk, bk, Wv, bv):
    out, _ = run(x, Wq, bq, Wk, bk, Wv, bv)
    return out


# revision 17
# speedup vs baseline: 1.0705x; 1.0705x over previous
"""Trainium2 Bass kernel for nn_Position_AM_Module_3D.

Reference computation (B=2, C=256, INTER=64, N=T*H*W=8192):
    q = Wq @ xf + bq          [B, N, 64]   (as Q [64, N] per batch)
    k = Wk @ xf + bk          [B, 64, N]
    v = Wv @ xf + bv          [B, 256, N]
    att = softmax(q @ k, -1)  [B, N, N]    (never materialized)
    out = v @ att^T + x       [B, 256, N]

Sharding: 8 cores = (batch b in 0..1) x (query quarter qc in 0..3).
Each core gets the full xf for its batch plus its 2048-query window, and
computes out[:, window] with a flash-attention-style k-loop:
  S^T tile [128k, 512q] = K_slice^T @ Q_chunk  (PE, f32r)
  e = exp(S^T)                                  (ACT; no max subtraction --
                                                 |energy| < ~60 << 88, safe in fp32)
  av[c,q]  += V^T_slice^T @ e                   (PE accumulate in PSUM)
  den[1,q] += ones^T @ e                        (PE accumulate in PSUM)
then out = av * (1/den) + x_window.
"""

import numpy as np

B, C, INTER = 2, 256, 64
T3, H3, W3 = 8, 32, 32
N = T3 * H3 * W3          # 8192
NCORES = 8
QCH = N // 4              # 2048 queries per core
QT = 512                  # q-chunk (one PSUM bank of fp32)
KT = 128                  # k-tile (PE partition dim)
NKT = N // KT             # 64 k-tiles
GK = 2                    # k-tiles per exp group ([128, 1024] ACTIVATE)
NG = NKT // GK            # 32 groups

_prog_cache = {}


def _build_program(mm_dtype_name="float32r", loop_n=1, variant=()):
    """variant: tuple of flags from
    {"phase_a_only", "no_den", "exp_on_dve", "ldw_probe"}."""
    from contextlib import ExitStack, nullcontext

    import concourse.bass as bass
    import concourse.mybir as mybir
    import concourse.tile as tile
    from concourse import bacc

    variant = set(variant)
    f32 = mybir.dt.float32
    mmdt = getattr(mybir.dt, mm_dtype_name)

    def mm(ap):
        return ap.bitcast(mmdt)

    nc = bacc.Bacc(None)

    xf = nc.declare_dram_parameter("xf", [C, N], f32, isOutput=False)
    xq = nc.declare_dram_parameter("xq", [C, QCH], f32, isOutput=False)
    wkT = nc.declare_dram_parameter("wkT", [C, INTER], f32, isOutput=False)
    wqT = nc.declare_dram_parameter("wqT", [C, INTER], f32, isOutput=False)
    wvT = nc.declare_dram_parameter("wvT", [C, C], f32, isOutput=False)
    bk = nc.declare_dram_parameter("bk", [INTER, 1], f32, isOutput=False)
    bq = nc.declare_dram_parameter("bq", [INTER, 1], f32, isOutput=False)
    bv = nc.declare_dram_parameter("bv", [1, C], f32, isOutput=False)
    out = nc.declare_dram_parameter("out", [C, QCH], f32, isOutput=True)

    Exp = mybir.ActivationFunctionType.Exp
    add_op = mybir.AluOpType.add

    with tile.TileContext(nc) as tc, ExitStack() as ctx:
        singles = ctx.enter_context(tc.tile_pool(name="singles", bufs=1))

        # persistent SBUF tensors
        wkT_sb = singles.tile([128, 2, INTER], f32)
        wqT_sb = singles.tile([128, 2, INTER], f32)
        wvT_sb = singles.tile([128, 2, C], f32)
        bk_sb = singles.tile([INTER, 1], f32)
        bq_sb = singles.tile([INTER, 1], f32)
        bv_bc = singles.tile([128, C], f32)
        ones_col = singles.tile([128, 1], f32)
        ones_row = singles.tile([1, 128], f32)
        K_sb = singles.tile([INTER, N], f32)
        Q_sb = singles.tile([INTER, QCH], f32)
        VT_sb = singles.tile([128, NKT, C], f32)
        xq_sb = singles.tile([128, 2, QCH], f32)

        # Memset can't write f32r; stage in f32 and round via DVE copy.
        ones_col_f = singles.tile([128, 1], f32)
        ones_row_f = singles.tile([1, 128], f32)
        nc.vector.memset(ones_col_f, 1.0)
        nc.vector.memset(ones_row_f, 1.0)
        nc.vector.tensor_copy(mm(ones_col), ones_col_f)
        nc.vector.tensor_copy(mm(ones_row), ones_row_f)

        def load_weights():
            for ci in range(2):
                # these buffers feed f32r matmuls: produce them as f32r
                # (bitwise copy) so the BIR verifier accepts the consumers
                nc.sync.dma_start(
                    out=mm(wkT_sb[:, ci, :]), in_=mm(wkT[ci * 128:(ci + 1) * 128, :])
                )
                nc.sync.dma_start(
                    out=mm(wqT_sb[:, ci, :]), in_=mm(wqT[ci * 128:(ci + 1) * 128, :])
                )
                nc.sync.dma_start(
                    out=mm(wvT_sb[:, ci, :]), in_=mm(wvT[ci * 128:(ci + 1) * 128, :])
                )
                nc.sync.dma_start(
                    out=mm(xq_sb[:, ci, :]), in_=mm(xq[ci * 128:(ci + 1) * 128, :])
                )
            nc.sync.dma_start(out=bk_sb, in_=bk[:, :])
            nc.sync.dma_start(out=bq_sb, in_=bq[:, :])
            # broadcast bv across partitions (DRAM src with partition step 0)
            bv_ap = bv[:, :]
            bv_src = bass.AP(
                tensor=bv_ap.tensor,
                offset=bv_ap.offset,
                ap=[[0, 128]] + list(bv_ap.ap[1:]),
            )
            nc.sync.dma_start(out=bv_bc, in_=bv_src)

        def phase_a(tag):
            # xf streams through SBUF in 1024-column chunks (both c-halves),
            # double-buffered so DMA overlaps the projection matmuls.
            CW = 1024
            with tc.tile_pool(name=f"xfp{tag}", bufs=2) as xfp, \
                 tc.tile_pool(name=f"ppsum{tag}", bufs=4, space="PSUM") as ppsum:
                for s in range(N // CW):
                    xt = xfp.tile([128, 2, CW], f32, tag="xf")
                    for ci in range(2):
                        nc.sync.dma_start(
                            out=mm(xt[:, ci, :]),
                            in_=mm(xf[ci * 128:(ci + 1) * 128, s * CW:(s + 1) * CW]),
                        )

                    # K = Wk @ xf + bk   -> K_sb [64, N]
                    for n2 in range(CW // 512):
                        pk = ppsum.tile([INTER, 512], f32, tag="pk")
                        for ci in range(2):
                            nc.tensor.matmul(
                                pk,
                                mm(wkT_sb[:, ci, :]),
                                mm(xt[:, ci, n2 * 512:(n2 + 1) * 512]),
                                start=(ci == 0),
                                stop=(ci == 1),
                            )
                        off = s * CW + n2 * 512
                        nc.vector.tensor_scalar(
                            mm(K_sb[:, off:off + 512]), pk, bk_sb, None, add_op
                        )

                    # V^T = xf^T @ Wv^T + bv -> VT_sb [128, jt, C]
                    for j2 in range(CW // KT):
                        pv = ppsum.tile([128, C], f32, tag="pv")
                        for ci in range(2):
                            nc.tensor.matmul(
                                pv,
                                mm(xt[:, ci, j2 * KT:(j2 + 1) * KT]),
                                mm(wvT_sb[:, ci, :]),
                                start=(ci == 0),
                                stop=(ci == 1),
                            )
                        jt = s * (CW // KT) + j2
                        nc.vector.tensor_tensor(
                            mm(VT_sb[:, jt, :]), pv, bv_bc, add_op
                        )

                # Q = Wq @ xq + bq -> Q_sb [64, QCH]
                for qt in range(QCH // 512):
                    pq = ppsum.tile([INTER, 512], f32, tag="pk")
                    for ci in range(2):
                        nc.tensor.matmul(
                            pq,
                            mm(wqT_sb[:, ci, :]),
                            mm(xq_sb[:, ci, qt * 512:(qt + 1) * 512]),
                            start=(ci == 0),
                            stop=(ci == 1),
                        )
                    nc.vector.tensor_scalar(
                        mm(Q_sb[:, qt * 512:(qt + 1) * 512]), pq, bq_sb, None, add_op
                    )

        def phase_b(tag):
            with tc.tile_pool(name=f"epool{tag}", bufs=2) as epool, \
                 tc.tile_pool(name=f"spsum{tag}", bufs=2, space="PSUM") as spsum, \
                 tc.tile_pool(name=f"avpsum{tag}", bufs=1, space="PSUM") as avpsum, \
                 tc.tile_pool(name=f"dpsum{tag}", bufs=1, space="PSUM") as dpsum, \
                 tc.tile_pool(name=f"bpsum{tag}", bufs=1, space="PSUM") as bpsum, \
                 tc.tile_pool(name=f"misc{tag}", bufs=2) as misc, \
                 tc.tile_pool(name=f"outp{tag}", bufs=4) as outp:

                for qc in range(QCH // QT):
                    q_rhs = mm(Q_sb[:, qc * QT:(qc + 1) * QT])
                    av = avpsum.tile([128, 2 * QT], f32, tag="av")
                    dps = dpsum.tile([1, QT], f32, tag="d")

                    def s_group(g):
                        sps = spsum.tile([128, GK * QT], f32, tag="s")
                        for t in range(GK):
                            kt = g * GK + t
                            if "ldw_probe" in variant:
                                nc.tensor.matmul(
                                    sps[:1, t * QT:(t + 1) * QT], mm(ones_col),
                                    q_rhs, start=True, stop=True)
                            else:
                                nc.tensor.matmul(
                                    sps[:, t * QT:(t + 1) * QT],
                                    mm(K_sb[:, kt * KT:(kt + 1) * KT]),
                                    q_rhs, start=True, stop=True)
                        e_t = epool.tile([128, GK * QT], f32, tag="e")
                        if "exp_on_dve" in variant:
                            nc.vector.tensor_copy(mm(e_t), sps)
                        else:
                            nc.scalar.activation(mm(e_t), sps, Exp)
                        return e_t

                    def av_group(g, e_t):
                        for t in range(GK):
                            kt = g * GK + t
                            er = mm(e_t[:, t * QT:(t + 1) * QT])
                            first = kt == 0
                            last = kt == NKT - 1
                            if "ldw_probe" in variant:
                                nc.tensor.matmul(av[:1, 0:QT], mm(ones_col), er,
                                                 start=first, stop=last)
                                nc.tensor.matmul(av[:1, QT:2 * QT], mm(ones_col),
                                                 er, start=first, stop=last)
                            else:
                                nc.tensor.matmul(av[:, 0:QT],
                                                 mm(VT_sb[:, kt, 0:128]), er,
                                                 start=first, stop=last)
                                nc.tensor.matmul(av[:, QT:2 * QT],
                                                 mm(VT_sb[:, kt, 128:256]), er,
                                                 start=first, stop=last)
                            if "no_den" not in variant:
                                nc.tensor.matmul(dps, mm(ones_col), er,
                                                 start=first, stop=last)

                    e_prev = s_group(0)
                    for g in range(NG):
                        e_next = s_group(g + 1) if g + 1 < NG else None
                        av_group(g, e_prev)
                        e_prev = e_next

                    # epilogue: out = av / den + xq
                    recip = misc.tile([1, QT], f32, tag="recip")
                    with nc.allow_low_precision(reason="f32r rounding of 1/den"):
                        nc.vector.reciprocal(mm(recip), dps)
                    # broadcast recip across partitions via PE outer product
                    bps = bpsum.tile([128, QT], f32, tag="b")
                    nc.tensor.matmul(bps, mm(ones_row), mm(recip),
                                     start=True, stop=True)
                    bsb = misc.tile([128, QT], f32, tag="bsb")
                    nc.vector.tensor_copy(bsb, bps)
                    for ch in range(2):
                        o_t = outp.tile([128, QT], f32, tag="o")
                        nc.vector.tensor_mul(o_t, av[:, ch * QT:(ch + 1) * QT], bsb)
                        nc.vector.tensor_add(
                            o_t, o_t, xq_sb[:, ch, qc * QT:(qc + 1) * QT]
                        )
                        nc.sync.dma_start(
                            out=out[ch * 128:(ch + 1) * 128, qc * QT:(qc + 1) * QT],
                            in_=o_t,
                        )

        load_weights()
        loop_cm = (lambda: tc.For_i(0, loop_n, 1)) if loop_n > 1 else nullcontext
        with loop_cm():
            phase_a("0")
            if "phase_a_only" not in variant:
                phase_b("0")

    nc.finalize()
    return nc


def _get_program(mm_dtype_name="float32r", loop_n=1, variant=()):
    key = (mm_dtype_name, loop_n, tuple(sorted(variant)))
    if key not in _prog_cache:
        _prog_cache[key] = _build_program(mm_dtype_name, loop_n, variant)
    return _prog_cache[key]


def _make_in_maps(x, Wq, bq, Wk, bk, Wv, bv):
    x = np.ascontiguousarray(np.asarray(x, np.float32))
    xf = x.reshape(B, C, N)
    wkT = np.ascontiguousarray(np.asarray(Wk, np.float32).T)
    wqT = np.ascontiguousarray(np.asarray(Wq, np.float32).T)
    wvT = np.ascontiguousarray(np.asarray(Wv, np.float32).T)
    bkc = np.ascontiguousarray(np.asarray(bk, np.float32).reshape(INTER, 1))
    bqc = np.ascontiguousarray(np.asarray(bq, np.float32).reshape(INTER, 1))
    bvc = np.ascontiguousarray(np.asarray(bv, np.float32).reshape(1, C))
    in_maps = []
    for core in range(NCORES):
        b, qc = divmod(core, NCORES // B)
        sl = slice(qc * QCH, (qc + 1) * QCH)
        in_maps.append({
            "xf": np.ascontiguousarray(xf[b]),
            "xq": np.ascontiguousarray(xf[b][:, sl]),
            "wkT": wkT, "wqT": wqT, "wvT": wvT,
            "bk": bkc, "bq": bqc, "bv": bvc,
        })
    return in_maps


def _gather(results):
    full = np.empty((B, C, N), np.float32)
    for core in range(NCORES):
        b, qc = divmod(core, NCORES // B)
        full[b][:, qc * QCH:(qc + 1) * QCH] = results[core]["out"]
    return full.reshape(B, C, T3, H3, W3)


def run(x, Wq, bq, Wk, bk, Wv, bv, trace=False, loop_n=1, variant=(),
        **spmd_kwargs):
    """Run the kernel; returns (output, BassKernelResults)."""
    from concourse.bass_utils import run_bass_kernel_spmd

    nc = _get_program(loop_n=loop_n, variant=variant)
    in_maps = _make_in_maps(x, Wq, bq, Wk, bk, Wv, bv)
    res = run_bass_kernel_spmd(
        nc, in_maps, list(range(NCORES)), trace=trace, **spmd_kwargs
    )
    return _gather(res.results), res


def kernel(x, Wq, bq, Wk, bk, Wv, bv):
    out, _ = run(x, Wq, bq, Wk, bk, Wv, bv)
    return out
